# revision 35
# baseline (speedup 1.0000x reference)
"""Trainium2 Bass kernel for nn_Conv1d_fft: polyphase Karatsuba, planar form.

The reference FFT conv is exactly a 129-tap cross-correlation with PAD=32:
    out[b,o,n] = bias[o] + sum_{i,t} w[o,i,t] * xp[b,i,n+t],  n in [0,4032)
with xp = x zero-padded to 4160. With v = taps-flipped w,
    out[n] = y[n + 128],   y = v (linear conv) xp   (length 4288).

Karatsuba polyphase splitting (P = v0*x0, Q = v1*x1, M = (v0+v1)(x0+x1);
y[2k] = P[k]+Q[k-1], y[2k+1] = M[k]-P[k]-Q[k]) cuts MACs by (3/4)^depth;
depth 4: 81 leaf convs of ~8 taps. Data-parallel over batch, 2 per core.

Design points (all measured on HW, 8-core SPMD):

* PLANAR COMBINE: every level keeps its signal as 2^l phase PLANES
  ([COUT, BPC, 2^l, W] tiles) instead of interleaving. In plane space
  every combine is a handful of wide CONTIGUOUS ops:
      y_pl[2r]   = P_pl[r] + Q_pl[r-1]     (r=0 wraps to Q_pl[np-1]
                                            shifted one column)
      y_pl[2r+1] = M_pl[r] - P_pl[r] - Q_pl[r]
  The elem-stride-2 interleave writes of the direct form ran at
  ~2.7ns/elem on DVE vs ~0.8 contiguous and made the combine tree the
  bottleneck (356 engine-us in a 203us kernel). The final interleave is
  a host-side numpy transpose: the device returns plane-major
  [BPC, COUT, 16, 252] fp16; host does .transpose(0,1,3,2) + fp32 cast.

* COLUMN TRIM: only leaf cols [J0-depth, J0+WOUT) = [4, 260) are ever
  consumed upward (the needed window shrinks one col per combine
  level), so leaves compute Wc = 256 cols, not s_leaf = 268. Bonus:
  256*BPC*4B = 2048B, so a fused-batch accumulator fits ONE PSUM bank:
  664 matmuls of [COUT, BPC, 256] (512 cols, ~220ns each, 3% off the
  fp16 stream roofline) instead of 1328 of 256.

* ENGINE POLICY: Vector and GpSimd slow each other ~3x when running
  concurrently (measured: identical 536-elem TTs, 430ns alone vs
  1.3us concurrent, even on disjoint tiles). GpSimd is left idle;
  Vector owns the combine tree + final; Scalar (no contention with
  Vector) takes PSUM drains, edge copies, and the P+bias precompute.

* TAIL: top-level subtrees emit M, P, Q (host layout permuted to
  match) and P' = P+b (scalar activation) / t' = (M+b)-P (vector stt)
  are precomputed while Q's leaves stream, so after the last matmul
  only Q's combine chain + 16 contiguous TTs + output DMA remain
  (~16us). stt costs 3.2ns/elem vs 0.8 for TT - keep stt off the tail.

* DMA: per-leaf DMAs cost ~640ns of Sync issue each (495 of them
  starved the Sync engine at depth 5); inputs are grouped [1,2,2,4]
  then 6 (w) / 9 (x) leaves per DMA, prefetched two groups ahead, and
  prefetch issues AFTER each leaf's matmuls so the first matmul isn't
  ordered behind later DMA issues. Out tiles are written at exactly
  window width so the out DMA reads contiguous 2016B runs (a strided
  fp16 source dribbled 488B packets at ~10GB/s/queue).

Measured: 174.2us, rel err 4.3e-3 (gate 2e-2). Baselines: direct conv
457.5us; interleaved Karatsuba depth-4 203.1us. Depth-5 variants lose:
combine volume grows 1.55x and the engines (not the PE) become the
wall at these per-elem rates.
"""

import os
import numpy as np

import concourse.bass as bass
import concourse.bacc as bacc
import concourse.tile as tile
import concourse.mybir as mybir
from concourse.bass_utils import run_bass_kernel_spmd

B, CIN, COUT, L, K = 16, 128, 128, 4096, 129
PAD = 32
OUT_LEN = 2 * PAD + L - (K - 1)   # 4032
LP = L + 2 * PAD                  # 4160
N_CORES = 8
BPC = B // N_CORES                # batches per core

F32 = mybir.dt.float32
F16 = mybir.dt.float16
ADD = mybir.AluOpType.add
SUB = mybir.AluOpType.subtract

# ---------------------------------------------------------------------------
# Karatsuba tree structure


def _leaf_taps(kv, depth):
    """Leaf tap-counts, traversal order [P-subtree, Q-subtree, M-subtree]."""
    if depth == 0:
        return [kv]
    k0 = (kv + 1) // 2
    return (_leaf_taps(k0, depth - 1)
            + _leaf_taps(kv // 2, depth - 1)
            + _leaf_taps(k0, depth - 1))


def _top_mpq(lst):
    """Reorder top-level subtree blocks [P,Q,M] -> [M,P,Q] (device emits
    M first so the tail after the last leaf is only Q's combine chain)."""
    n = len(lst) // 3
    return lst[2 * n:] + lst[:n] + lst[n:2 * n]


class _Plan:
    def __init__(self, depth):
        self.depth = depth
        self.leaf_taps = _top_mpq(_leaf_taps(K, depth))
        self.nbr = len(self.leaf_taps)
        kmax = K
        for _ in range(depth):
            kmax = (kmax + 1) // 2
        self.kmax = kmax
        self.xleaf = LP >> depth
        self.xpadl = kmax - 1
        self.xbuf = self.xleaf + 2 * self.xpadl
        self.s_leaf = self.xleaf + kmax - 1     # plane width W
        self.tot_taps = sum(self.leaf_taps)
        self.NP = 1 << depth                    # final plane count
        assert (K - 1) % self.NP == 0
        self.J0 = (K - 1) // self.NP            # out plane col offset
        assert OUT_LEN % self.NP == 0
        self.WOUT = OUT_LEN // self.NP          # out plane width
        def gsizes(total, body):
            lead = [1, 2, 2, 4]
            if total <= 9 or (total - 9) % body != 0:
                return [total]
            return lead + [body] * ((total - 9) // body)
        self.wg_sizes = gsizes(self.nbr, 6)
        self.xg_sizes = gsizes(self.nbr, 9)
        # leaf -> (group, index-within-group)
        def l2g(sizes):
            m = []
            for g, s in enumerate(sizes):
                for j in range(s):
                    m.append((g, j))
            return m
        self.leaf_wg = l2g(self.wg_sizes)
        self.leaf_xg = l2g(self.xg_sizes)
        # leaf -> start-leaf index of its x group
        starts = []
        acc = 0
        for s in self.xg_sizes:
            starts.append(acc)
            acc += s
        self.xg_start = starts
        self.trim0 = self.J0 - depth          # left trim of leaf outputs
        assert self.trim0 >= 0
        self.Wc = self.WOUT + depth           # computed plane width
        # fused-batch leaf accumulator if both batches fit one PSUM bank
        self.fused_leaf = self.Wc * BPC * 4 <= 2048
        assert self.Wc * 4 <= 2048, "per-batch acc must fit PSUM bank"


def _leaf_weight_list(v, depth):
    if depth == 0:
        return [v]
    v0 = v[:, :, 0::2]
    v1 = v[:, :, 1::2]
    v1p = v1
    if v1.shape[-1] < v0.shape[-1]:
        v1p = np.pad(v1, ((0, 0), (0, 0), (0, v0.shape[-1] - v1.shape[-1])))
    return (_leaf_weight_list(v0, depth - 1)
            + _leaf_weight_list(v1, depth - 1)
            + _leaf_weight_list(v0 + v1p, depth - 1))


def _leaf_x_list(x, depth):
    if depth == 0:
        return [x]
    x0 = x[..., 0::2]
    x1 = x[..., 1::2]
    return (_leaf_x_list(x0, depth - 1)
            + _leaf_x_list(x1, depth - 1)
            + _leaf_x_list(x0 + x1, depth - 1))


# ---------------------------------------------------------------------------
# Device program

_cache = {}


def _build_program(depth):
    pl = _Plan(depth)
    W = pl.Wc                 # computed plane width (trimmed)
    NP = pl.NP
    nc = bacc.Bacc("TRN2", target_bir_lowering=False, debug=False,
                   num_devices=N_CORES)

    x_d = nc.dram_tensor("x", [CIN, BPC, pl.nbr * pl.xbuf], F16,
                         kind="ExternalInput").ap()
    w_d = nc.dram_tensor("w", [CIN, pl.tot_taps * COUT], F16,
                         kind="ExternalInput").ap()
    b_d = nc.dram_tensor("b", [COUT, 1], F32, kind="ExternalInput").ap()
    # planar output: plane-major [BPC, COUT, NP, WOUT]; host de-interleaves
    # (pure layout transpose) to [BPC, COUT, OUT_LEN]
    o_d = nc.dram_tensor("out", [BPC, COUT, NP, pl.WOUT], F16,
                         kind="ExternalOutput").ap()

    n_xg = len(pl.xg_sizes)
    n_wg = len(pl.wg_sizes)
    wg_tap0 = []
    t = 0
    li0 = 0
    for g in range(n_wg):
        wg_tap0.append(t)
        t += sum(pl.leaf_taps[li0:li0 + pl.wg_sizes[g]])
        li0 += pl.wg_sizes[g]
    wg_tap0.append(t)

    with tile.TileContext(nc) as tc:
        from contextlib import ExitStack
        es = ExitStack()
        with es:
            wp = es.enter_context(tc.tile_pool(name="wp", bufs=3))
            xpool = es.enter_context(tc.tile_pool(name="xp", bufs=3))
            bp = es.enter_context(tc.tile_pool(name="bp", bufs=1))
            lf = es.enter_context(tc.tile_pool(name="lf", bufs=10))
            ps = es.enter_context(
                tc.tile_pool(name="ps", bufs=8, space=bass.MemorySpace.PSUM))
            ypools = {}
            tpools = {}
            ybufs = {1: 6, 2: 5, 3: 3, 4: 3}
            for lvl in range(1, depth):
                ypools[lvl] = es.enter_context(
                    tc.tile_pool(name=f"y{lvl}", bufs=ybufs.get(lvl, 4)))
                tpools[lvl] = es.enter_context(
                    tc.tile_pool(name=f"t{lvl}", bufs=2))
            tpools[depth] = es.enter_context(
                tc.tile_pool(name=f"t{depth}", bufs=2))
            ocp = es.enter_context(tc.tile_pool(name="oc", bufs=3))
            prep = es.enter_context(tc.tile_pool(name="pre", bufs=1))

            wg_tiles = [None] * n_wg
            xg_tiles = [None] * n_xg
            b_sb = bp.tile([COUT, 1], F32, name="bsb")

            def fetch_wg(g):
                gt = wg_tap0[g + 1] - wg_tap0[g]
                wt = wp.tile([CIN, gt * COUT], F16, tag="w", name=f"wg{g}")
                nc.sync.dma_start(
                    wt[:], w_d[:, wg_tap0[g] * COUT:wg_tap0[g + 1] * COUT])
                wg_tiles[g] = wt

            def fetch_xg(g):
                sz = pl.xg_sizes[g]
                s0 = pl.xg_start[g]
                xt = xpool.tile([CIN, BPC, sz * pl.xbuf], F16, tag="x",
                                name=f"xg{g}")
                nc.sync.dma_start(
                    xt[:], x_d[:, :, s0 * pl.xbuf:(s0 + sz) * pl.xbuf])
                xg_tiles[g] = xt

            fetch_wg(0)
            fetch_xg(0)
            nc.sync.dma_start(b_sb[:], b_d[:])

            leaf_idx = [0]
            tap_off = [0]

            def emit_leaf():
                li = leaf_idx[0]
                leaf_idx[0] += 1
                kb = pl.leaf_taps[li]
                t0 = tap_off[0]
                tap_off[0] += kb

                wg, wj = pl.leaf_wg[li]
                xg, xj = pl.leaf_xg[li]
                w_sb = wg_tiles[wg]
                wt0 = (t0 - wg_tap0[wg]) * COUT
                x_sb = xg_tiles[xg]
                xq0 = xj * pl.xbuf

                lt = lf.tile([COUT, BPC, 1, W], F16, tag="leaf",
                             name=f"leaf{li}")
                if pl.fused_leaf:
                    acc = ps.tile([COUT, BPC, W], F32, tag="acc",
                                  name=f"acc{li}")
                    for s in range(kb):
                        w_ap = w_sb[:, wt0 + s * COUT:wt0 + (s + 1) * COUT]
                        off = xq0 + pl.xpadl + pl.trim0 - s
                        nc.tensor.matmul(
                            acc[:], w_ap, x_sb[:, :, off:off + W],
                            start=(s == 0), stop=(s == kb - 1),
                        )
                    if li % 2 == 0:
                        nc.scalar.copy(lt[:, :, 0, :], acc[:])
                    else:
                        nc.vector.tensor_copy(lt[:, :, 0, :], acc[:])
                else:
                    accs = [ps.tile([COUT, W], F32, tag="acc",
                                    name=f"acc{li}_{bt}")
                            for bt in range(BPC)]
                    for s in range(kb):
                        w_ap = w_sb[:, wt0 + s * COUT:wt0 + (s + 1) * COUT]
                        off = xq0 + pl.xpadl + pl.trim0 - s
                        for bt in range(BPC):
                            nc.tensor.matmul(
                                accs[bt][:],
                                w_ap,
                                x_sb[:, bt, off:off + W],
                                start=(s == 0), stop=(s == kb - 1),
                            )
                    for bt in range(BPC):
                        if (li * BPC + bt) % 2 == 0:
                            nc.scalar.copy(lt[:, bt, 0, :], accs[bt][:])
                        else:
                            nc.vector.tensor_copy(lt[:, bt, 0, :],
                                                  accs[bt][:])
                # prefetch (after matmul emission so the first matmuls
                # don't order behind later DMA issues on Sync)
                for g in (wg + 1, wg + 2):
                    if wj == 0 and g < n_wg and wg_tiles[g] is None:
                        fetch_wg(g)
                for g in (xg + 1, xg + 2):
                    if xj == 0 and g < n_xg and xg_tiles[g] is None:
                        fetch_xg(g)
                return lt

            node_ctr = [0]

            def combine(p, q, m, np_, lvl):
                """p/q/m: [COUT, BPC, np_, W] -> y: [COUT, BPC, 2np_, W]."""
                ni = node_ctr[0]
                node_ctr[0] += 1
                y = ypools[lvl].tile([COUT, BPC, 2 * np_, W], F16,
                                     tag=f"y{lvl}", name=f"y{lvl}_{ni}")
                t = tpools[lvl].tile([COUT, BPC, np_, W], F16,
                                     tag=f"t{lvl}", name=f"tc{lvl}_{ni}")
                # evens r>=1 (contiguous reads, plane-stride-2 writes)
                if np_ > 1:
                    nc.vector.tensor_add(y[:, :, 2:2 * np_:2, :],
                                         p[:, :, 1:np_, :],
                                         q[:, :, 0:np_ - 1, :])
                # even r=0: y0[1:] = P0[1:] + Q[np-1][:-1]; y0[0] = P0[0]
                nc.vector.tensor_add(y[:, :, 0, 1:W], p[:, :, 0, 1:W],
                                     q[:, :, np_ - 1, 0:W - 1])
                nc.scalar.copy(y[:, :, 0, 0:1], p[:, :, 0, 0:1])
                # t = M - P ; odds = t - Q   (vector: gpsimd is ~2.7ns/elem)
                nc.vector.tensor_sub(t[:], m[:], p[:])
                nc.vector.tensor_sub(y[:, :, 1::2, :], t[:], q[:])
                return y

            def precompute_final(p, m):
                # mid-run (during Q subtree): P' = P + b  (scalar act),
                # t' = (M + b) - P  (vector stt).  Tail then needs only
                # cheap TTs:  even = P'[r] + Q[r-1],  odd = t'[r] - Q[r].
                np_ = NP // 2
                wo = pl.WOUT
                js = pl.J0
                wo = pl.WOUT
                js = depth
                pp = prep.tile([COUT, BPC, np_, wo + 1], F16, tag="pp",
                               name="ppre")
                tp = prep.tile([COUT, BPC, np_, wo], F16, tag="tp",
                               name="tpre")
                ident = mybir.ActivationFunctionType.Identity
                for bt in range(BPC):
                    # pp holds window [js-1, js+wo): one extra left col so
                    # the r=0 even (Q[np-1] shifted) also reads contiguous
                    nc.scalar.activation(pp[:, bt],
                                         p[:, bt, :, js - 1:js + wo],
                                         ident, bias=b_sb[:])
                    for h in range(2):
                        hs = np_ // 2
                        nc.vector.scalar_tensor_tensor(
                            tp[:, bt, h * hs:(h + 1) * hs, :],
                            m[:, bt, h * hs:(h + 1) * hs, js:js + wo],
                            b_sb[:],
                            p[:, bt, h * hs:(h + 1) * hs, js:js + wo],
                            ADD, SUB)
                return pp, tp

            def emit_final(pp, tp, q, p_unused=None):
                # out planes: out_pl[r][j] = y_pl[r][j + J0].  Chunked by
                # PLANES (4 per chunk): contiguous 4*WOUT*2B DMA runs.
                np_ = NP // 2
                wo = pl.WOUT
                js = depth                   # J0 in trimmed coordinates
                nch = 4
                pc = NP // nch               # planes per chunk
                hp = pc // 2                 # even (or odd) planes per chunk
                for c in range(nch):
                    r0 = hp * c              # even-plane r range [r0, r0+hp)
                    oc = ocp.tile([COUT, BPC, pc, wo], F16, tag="oc",
                                  name=f"oc{c}")
                    # window-width oc: out DMA source is fully contiguous
                    # (2016B runs vs 488B packets at 10GB/s/queue)
                    for bt in range(BPC):
                        if c == 0:
                            # r=0 reads pp's extra left col (Q[np-1] shift)
                            nc.vector.tensor_add(
                                oc[:, bt, 0, :], pp[:, bt, 0, 1:wo + 1],
                                q[:, bt, np_ - 1, js - 1:js - 1 + wo])
                            nc.vector.tensor_add(
                                oc[:, bt, 2, :], pp[:, bt, 1, 1:wo + 1],
                                q[:, bt, 0, js:js + wo])
                        else:
                            nc.vector.tensor_add(
                                oc[:, bt, 0:pc:2, :],
                                pp[:, bt, r0:r0 + hp, 1:wo + 1],
                                q[:, bt, r0 - 1:r0 - 1 + hp, js:js + wo])
                        nc.vector.tensor_sub(oc[:, bt, 1:pc:2, :],
                                             tp[:, bt, r0:r0 + hp, :],
                                             q[:, bt, r0:r0 + hp, js:js + wo])
                    for bt in range(BPC):
                        nc.sync.dma_start(
                            o_d[bt][:, c * pc:(c + 1) * pc, :],
                            oc[:, bt, :, :])

            def emit(d):
                if d == 0:
                    return emit_leaf()
                if d == depth:
                    # M first, then P, then Q: the tail after the last
                    # leaf only runs Q's combine chain + cheap final TTs
                    m = emit(d - 1)
                    p = emit(d - 1)
                    pp, tp = precompute_final(p, m)
                    q = emit(d - 1)
                    emit_final(pp, tp, q)
                    return None
                p = emit(d - 1)
                q = emit(d - 1)
                m = emit(d - 1)
                return combine(p, q, m, 1 << (d - 1), d)

            emit(depth)

    nc.compile()
    return nc


def _get_program(depth):
    if depth not in _cache:
        _cache[depth] = _build_program(depth)
    return _cache[depth]


def kernel(x, weight, bias, _trace=False, _trace_kwargs=None):
    depth = int(os.environ.get("BASS_KARA_DEPTH", "4"))
    pl = _Plan(depth)
    nc = _get_program(depth)

    xp_full = np.zeros((B, CIN, LP), dtype=np.float32)
    xp_full[:, :, PAD:PAD + L] = np.asarray(x, dtype=np.float32)
    v = np.ascontiguousarray(np.asarray(weight, dtype=np.float32)[:, :, ::-1])

    xl = _top_mpq(_leaf_x_list(xp_full, depth))
    xbuf = np.zeros((B, CIN, pl.nbr * pl.xbuf), dtype=np.float16)
    for li, a in enumerate(xl):
        xbuf[:, :, li * pl.xbuf + pl.xpadl:
             li * pl.xbuf + pl.xpadl + pl.xleaf] = a.astype(np.float16)
    xbuf = np.ascontiguousarray(np.transpose(
        xbuf.reshape(N_CORES, BPC, CIN, -1), (0, 2, 1, 3)))

    wl = _top_mpq(_leaf_weight_list(v, depth))
    wcat = np.concatenate(
        [np.transpose(a, (1, 2, 0)).reshape(CIN, -1) for a in wl], axis=1)
    wcat = np.ascontiguousarray(wcat.astype(np.float16))
    assert wcat.shape == (CIN, pl.tot_taps * COUT)

    b2 = np.ascontiguousarray(np.asarray(bias, np.float32).reshape(COUT, 1))

    in_maps = [
        {"x": xbuf[c], "w": wcat, "b": b2}
        for c in range(N_CORES)
    ]
    res = run_bass_kernel_spmd(
        nc, in_maps, list(range(N_CORES)),
        trace=_trace, **(_trace_kwargs or {}),
    )
    od = np.concatenate([res.results[c]["out"] for c in range(N_CORES)],
                        axis=0)          # (B, COUT, NP, WOUT) f16
    out = np.ascontiguousarray(od.transpose(0, 1, 3, 2)).reshape(
        B, COUT, OUT_LEN).astype(np.float32)
    if _trace:
        return out, res
    return out
